# revision 29
# baseline (speedup 1.0000x reference)
"""Trainium2 Bass kernel for nn_Block_73615739454081 (tri-block sparse attention + FFN).

Contract: kernel(**inputs) takes FULL unsharded inputs (as produced by
setup_inputs()) and returns the FULL [1, N, D] float32 output.

Strategy (8 NeuronCores, SPMD):
  - Shard the block axis: 161 blocks of 256 tokens. Each core runs an identical
    program over 21 "local" blocks plus a 1-block halo on each side (23-block
    slab). Adjacent cores overlap by one block; the gather keeps a disjoint
    20/20/.../21 split. No collectives: the halo is materialized host-side.
  - Conditioning (scale/offset from global_norm_conditioning) and the attention
    1/sqrt(d) scale are folded into the weight matrices on the host, so the
    device only computes plain LayerNorm stats.
  - Attention is computed in transposed (feature-major) layouts throughout:
    qT/kT = [HD, tokens], scores ST = [keys, q], so no transposes are needed
    inside the attention core. Softmax skips max-subtraction (logits are O(1));
    denominators come from ones-matmuls, broadcast back via a K=1 matmul.
  - Key-validity masking (padding tokens / missing neighbor blocks) is data
    driven: exp(scores) rows are multiplied by a per-core 0/1 key-mask at the
    (statically known) slab edges.
  - All matmuls run in bf16 (fp32 "HIGH" mode streams at ~half rate and
    disables fast weight load); accumulation stays fp32 in PSUM, and the
    residual path stays fp32 in SBUF.
  - Each chain's softmax-normalize + projection + FFN is emitted after the
    NEXT block's QKV matmuls so the [1,qw] reciprocal latency hides behind
    independent tensor-engine work (keeps the PE HAM clock warm).
"""

import numpy as np

# ---------------------------------------------------------------- constants
N = 40962
D = 512
H = 4
HD = 128
BS = 256
NB = 161
NP = NB * BS
C = 16
FFW = 4 * D
SCALE = HD ** -0.5
EPS = 1e-5

NCORES = 8
LOCAL = 21                 # local blocks per core (uniform SPMD program)
SLAB = LOCAL + 2           # +1 halo block each side
SNODES = SLAB * BS         # 5888 slab tokens
KT = SNODES // 128         # 46 key tiles of 128 in the slab
OUT_NODES = LOCAL * BS     # 5376
STARTS = [0, 20, 40, 60, 80, 100, 120, 140]   # first local block per core
# kt tiles that can contain invalid keys on some core (slab edges):
FIXUP_KTS = (0, 1, KT - 4, KT - 3, KT - 2, KT - 1)

_PROG_CACHE = {}


# ---------------------------------------------------------------- device code
def _build_program(has_bf, has_bd, debug_stage=None, repeat=1):
    import concourse.bass as bass  # noqa: F401
    import concourse.mybir as mybir
    import concourse.tile as tile
    from concourse import bacc

    F32 = mybir.dt.float32
    BF16 = mybir.dt.bfloat16
    AF = mybir.ActivationFunctionType
    OP = mybir.AluOpType

    nc = bacc.Bacc("TRN2", target_bir_lowering=False, debug=False)

    def din(name, shape, dt):
        return nc.dram_tensor(name, shape, dt, kind="ExternalInput").ap()

    x_d = din("x_slab", [SNODES, D], F32)
    km_d = din("kmask", [128, KT], F32)
    wqT_d = din("wqT", [128, 4 * H * HD], BF16)
    wkT_d = din("wkT", [128, 4 * H * HD], BF16)
    wvN_d = din("wvN", [128, 4 * D], BF16)
    wfN_d = din("wfN", [128, H * D], BF16)
    wupT_d = din("wupT", [128, 4 * FFW], BF16)
    wdnN_d = din("wdnN", [128, 16 * D], BF16)
    cqB_d = din("cqB", [128, H], F32)
    ckB_d = din("ckB", [128, H], F32)
    cvB_d = din("cvB", [128, D], F32)
    cuB_d = din("cuB", [128, 16], F32)
    ident_d = din("ident", [128, 128], BF16)
    epsB_d = din("epsB", [128, 1], F32)
    if has_bf:
        bfB_d = din("bfB", [128, D], F32)
    if has_bd:
        bdB_d = din("bdB", [128, D], F32)
    out_d = nc.dram_tensor("out", [OUT_NODES, D], F32, kind="ExternalOutput").ap()

    with nc.allow_low_precision(reason="bf16 matmul operands by design"), \
         tile.TileContext(nc) as tc:
        with (
            tc.tile_pool(name="wconst", bufs=1) as wpool,
            tc.tile_pool(name="sb", bufs=2) as sb,
            tc.tile_pool(name="ps", bufs=2, space="PSUM") as ps,
        ):
            # ---------------- resident weights / constants
            # small constants first, then weights in first-use order so the
            # first phase1 blocks only on a small DMA prefix.
            epsB = wpool.tile([128, 1], F32, name="epsB")
            nc.scalar.dma_start(out=epsB, in_=epsB_d)
            ident = wpool.tile([128, 128], BF16, name="ident")
            nc.scalar.dma_start(out=ident, in_=ident_d)
            cqB = wpool.tile([128, H], F32, name="cqB")
            nc.scalar.dma_start(out=cqB, in_=cqB_d)
            ckB = wpool.tile([128, H], F32, name="ckB")
            nc.scalar.dma_start(out=ckB, in_=ckB_d)
            cvB = wpool.tile([128, D], F32, name="cvB")
            nc.scalar.dma_start(out=cvB, in_=cvB_d)
            cuB = wpool.tile([128, 16], F32, name="cuB")
            nc.scalar.dma_start(out=cuB, in_=cuB_d)
            kmt = wpool.tile([128, KT], F32, name="kmt")
            nc.scalar.dma_start(out=kmt, in_=km_d)
            if has_bf:
                bfB = wpool.tile([128, D], F32, name="bfB")
                nc.scalar.dma_start(out=bfB, in_=bfB_d)
            if has_bd:
                bdB = wpool.tile([128, D], F32, name="bdB")
                nc.scalar.dma_start(out=bdB, in_=bdB_d)
            wkT = wpool.tile([128, 4 * H * HD], BF16, name="wkT")
            nc.scalar.dma_start(out=wkT, in_=wkT_d)
            wvN = wpool.tile([128, 4 * D], BF16, name="wvN")
            nc.scalar.dma_start(out=wvN, in_=wvN_d)
            wqT = wpool.tile([128, 4 * H * HD], BF16, name="wqT")
            nc.scalar.dma_start(out=wqT, in_=wqT_d)
            wfN = wpool.tile([128, H * D], BF16, name="wfN")
            nc.scalar.dma_start(out=wfN, in_=wfN_d)
            wupT = wpool.tile([128, 4 * FFW], BF16, name="wupT")
            nc.scalar.dma_start(out=wupT, in_=wupT_d)
            wdnN = wpool.tile([128, 16 * D], BF16, name="wdnN")
            nc.scalar.dma_start(out=wdnN, in_=wdnN_d)

            kT_ring = {}
            v_ring = {}
            qT_ring = {}

            def layer_norm(src, nm, apply_on_act=False):
                """LN stats + apply: returns hn tile [128, 512] bf16."""
                stats = sb.tile([128, 6], F32, tag="stats", bufs=4, name=f"st{nm}")
                nc.vector.bn_stats(out=stats, in_=src)
                mv = sb.tile([128, 2], F32, tag="mv", bufs=4, name=f"mv{nm}")
                nc.vector.bn_aggr(out=mv, in_=stats)
                std = sb.tile([128, 1], F32, tag="std", bufs=4, name=f"sd{nm}")
                nc.scalar.activation(std, mv[:, 1:2], AF.Sqrt, bias=epsB)
                rstd = sb.tile([128, 1], F32, tag="rstd", bufs=4, name=f"rs{nm}")
                nc.vector.reciprocal(rstd, std)
                hn = sb.tile([128, D], BF16, tag="hn", bufs=8, name=f"hn{nm}")
                if apply_on_act:
                    # out = rstd*x - mu*rstd on ACT (Identity is in every
                    # table set) -> shortens the DVE serial chain in LN2.
                    nmr = sb.tile([128, 1], F32, tag="nmr", bufs=4,
                                  name=f"nm{nm}")
                    nc.vector.tensor_scalar(
                        nmr, mv[:, 0:1], rstd, -1.0, OP.mult, OP.mult
                    )
                    nc.scalar.activation(hn, src, AF.Identity, bias=nmr,
                                         scale=rstd)
                else:
                    nc.vector.tensor_scalar(
                        hn, src, mv[:, 0:1], rstd, OP.subtract, OP.mult
                    )
                return hn

            def phase1(j):
                """LN1 + transpose + q/k/v projections for slab block j."""
                r0 = j * BS
                hn = []
                for nt in range(2):
                    xt = sb.tile([128, D], F32, tag="xin", bufs=4, name=f"x{j}_{nt}")
                    nc.sync.dma_start(out=xt, in_=x_d[r0 + nt * 128:r0 + nt * 128 + 128, :])
                    hn.append(layer_norm(xt, f"1_{j}_{nt}"))
                # transpose hn -> h1T [128, 4*256] (d-tile-major, token minor)
                h1T = sb.tile([128, 4 * BS], BF16, tag="h1T", bufs=2, name=f"h1T{j}")
                for g in range(2):  # two psum tiles, each covers 2 d-tiles
                    pT = ps.tile([128, 512], BF16, tag="p1", bufs=2, name=f"pT{j}_{g}")
                    for dl in range(2):
                        dt = g * 2 + dl
                        for nt in range(2):
                            nc.tensor.transpose(
                                pT[:, dl * 256 + nt * 128:dl * 256 + nt * 128 + 128],
                                hn[nt][:, dt * 128:dt * 128 + 128],
                                ident,
                            )
                    if g == 0:
                        nc.vector.tensor_copy(h1T[:, 0:512], pT)
                    else:
                        nc.scalar.copy(h1T[:, 512:1024], pT)

                def proj_T(wT, cB, nm):
                    """qT/kT-style projection -> [128, H*256] bf16 tile."""
                    outt = sb.tile([128, H * BS], BF16, tag=f"{nm}ring",
                                   bufs=(3 if nm == "q" else 4), name=f"{nm}T{j}")
                    for g in range(2):
                        pQ = ps.tile([128, 512], F32, tag="p1", bufs=2,
                                     name=f"p{nm}{j}_{g}")
                        for hl in range(2):
                            h = g * 2 + hl
                            for kt in range(4):
                                nc.tensor.matmul(
                                    pQ[:, hl * 256:hl * 256 + 256],
                                    lhsT=wT[:, (kt * H + h) * 128:(kt * H + h) * 128 + 128],
                                    rhs=h1T[:, kt * 256:kt * 256 + 256],
                                    start=(kt == 0), stop=(kt == 3),
                                )
                        # bias-add + copy to sbuf, split DVE/ACT
                        for hl in range(2):
                            h = g * 2 + hl
                            dst = outt[:, h * 256:h * 256 + 256]
                            src = pQ[:, hl * 256:hl * 256 + 256]
                            if hl == 0:
                                nc.vector.tensor_scalar(
                                    dst, src, cB[:, h:h + 1], None, OP.add
                                )
                            else:
                                nc.scalar.add(dst, src, cB[:, h:h + 1])
                    return outt

                if 1 <= j <= LOCAL:
                    qT_ring[j] = proj_T(wqT, cqB, "q")
                kT_ring[j] = proj_T(wkT, ckB, "k")
                for nt in range(2):
                    pV = ps.tile([128, 512], F32, tag="p1", bufs=2, name=f"pV{j}_{nt}")
                    for kt in range(4):
                        nc.tensor.matmul(
                            pV,
                            lhsT=h1T[:, kt * 256 + nt * 128:kt * 256 + nt * 128 + 128],
                            rhs=wvN[:, kt * D:kt * D + D],
                            start=(kt == 0), stop=(kt == 3),
                        )
                    # v_aug layout: per-head 129-wide slot [ones | v_h] so the
                    # AV matmul's rhs yields the softmax denominator in col 0.
                    vt = sb.tile([128, H * (HD + 1)], BF16, tag="vring", bufs=8,
                                 name=f"v{j}_{nt}")
                    nc.gpsimd.memset(vt, 1.0)
                    for h in range(H):
                        nc.vector.tensor_tensor(
                            vt[:, h * 129 + 1:h * 129 + 129],
                            pV[:, h * 128:h * 128 + 128],
                            cvB[:, h * 128:h * 128 + 128], OP.add)
                    v_ring[(j, nt)] = vt

            def chain_attn(b0, nb):
                """Attention for local blocks b0..b0+nb-1.

                Scores stay [keys, q]; the AV matmul uses exp(S) as the
                STATIONARY operand against the augmented V ([ones | v_h]
                slots), producing o = [q, 1 + hd] per query subtile with the
                softmax denominator in column 0.  Normalization is then a
                [128,1] reciprocal + per-partition scale (DVE-only, so the
                tensor engine never waits on it), and a PE transpose brings
                o back to [hd, q] for the final projection.  Returns
                (oTn, finisher); the caller emits finisher() later so the
                last head's transposes hide behind independent matmuls.
                """
                qw = nb * BS
                nqs = 2 * nb
                kts = list(range(2 * b0, 2 * b0 + 2 * (nb + 2)))

                def _vbis(kt):
                    return [bi for bi in range(nb)
                            if 2 * (b0 + bi) <= kt <= 2 * (b0 + bi) + 5]

                # prefetch the residual rows rest_A will need, so the adds
                # there never wait on DMA latency
                xres_t = []
                for qs in range(nqs):
                    xres = sb.tile([128, D], F32, tag="xres", bufs=8,
                                   name=f"xr{b0}_{qs}")
                    r0 = (b0 + 1) * BS + qs * 128
                    nc.sync.dma_start(out=xres, in_=x_d[r0:r0 + 128, :])
                    xres_t.append(xres)

                oTn = []
                pend = None
                for h in range(H):
                    # ---- scores + exp for every kt of this head
                    Et = {}
                    for kt in kts:
                        vbis = _vbis(kt)
                        c0 = min(vbis) * BS
                        c1 = (max(vbis) + 1) * BS
                        STp = ps.tile([128, qw], F32, tag="st", bufs=2,
                                      name=f"S{b0}_{h}_{kt}")
                        ksrc = kT_ring[kt // 2][:, h * 256 + (kt % 2) * 128:
                                                h * 256 + (kt % 2) * 128 + 128]
                        for bi in vbis:
                            nc.tensor.matmul(
                                STp[:, bi * BS:bi * BS + BS],
                                lhsT=ksrc,
                                rhs=qT_ring[b0 + bi + 1][:, h * 256:h * 256 + 256],
                                start=True, stop=True,
                            )
                        E = sb.tile([128, qw], BF16, tag="E", bufs=10,
                                    name=f"E{b0}_{h}_{kt}")
                        nc.scalar.activation(E[:, c0:c1], STp[:, c0:c1], AF.Exp)
                        if kt in FIXUP_KTS:
                            nc.vector.tensor_scalar(
                                E[:, c0:c1], E[:, c0:c1],
                                kmt[:, kt:kt + 1], None, OP.mult
                            )
                        Et[kt] = E
                    if pend is not None:
                        pend()      # prev head's transposes, covered by scores
                        pend = None
                    # ---- AV': per query subtile, accumulate over its 6 kts
                    oah = [ps.tile([128, 2 * (HD + 1)], F32, tag="oacc", bufs=2,
                                   name=f"oa{b0}_{h}_{hf}") for hf in range(nb)]
                    for qs in range(nqs):
                        bi = qs // 2
                        myk = list(range(2 * (b0 + bi), 2 * (b0 + bi) + 6))
                        dst = oah[qs // 2][:, (qs % 2) * 129:(qs % 2) * 129 + 129]
                        for i, kt in enumerate(myk):
                            nc.tensor.matmul(
                                dst,
                                lhsT=Et[kt][:, qs * 128:qs * 128 + 128],
                                rhs=v_ring[(kt // 2, kt % 2)][:, h * 129:h * 129 + 129],
                                start=(i == 0), stop=(i == 5),
                            )
                    # ---- normalize on DVE (off the PE critical path)
                    oos = []
                    for qs in range(nqs):
                        src = oah[qs // 2][:, (qs % 2) * 129:(qs % 2) * 129 + 129]
                        rdn = sb.tile([128, 1], F32, tag="rdn", bufs=8,
                                      name=f"rd{b0}_{h}_{qs}")
                        nc.vector.reciprocal(rdn, src[:, 0:1])
                        oo = sb.tile([128, HD], BF16, tag="oon", bufs=8,
                                     name=f"oo{b0}_{h}_{qs}")
                        nc.vector.tensor_scalar(oo, src[:, 1:129], rdn, None,
                                                OP.mult)
                        oos.append(oo)
                    oT = sb.tile([128, qw], BF16, tag="oTn", bufs=8,
                                 name=f"oT{b0}_{h}")

                    def mk(h=h, oos=oos, oT=oT, nqs=nqs, qw=qw, b0=b0):
                        trp = ps.tile([128, qw], BF16, tag="p1", bufs=2,
                                      name=f"tr{b0}_{h}")
                        for qs in range(nqs):
                            nc.tensor.transpose(
                                trp[:, qs * 128:qs * 128 + 128], oos[qs], ident)
                        if h % 2 == 0:
                            nc.vector.tensor_copy(oT, trp)
                        else:
                            nc.scalar.copy(oT, trp)
                    pend = mk
                    oTn.append(oT)
                return (oTn, xres_t), pend

            def rest_A(b0, nb, oTn, xres_t):
                """Final projection + residual + LN2 for local blocks.

                The residual is folded into the PSUM accumulation via an
                identity matmul, so LN2 stats/apply read the PSUM tile
                directly and the DVE serial chain per subtile is minimal.
                """
                qw = nb * BS
                nqs = qw // 128
                r1 = []
                hn2 = []
                for qs in range(nqs):
                    at = ps.tile([128, D], F32, tag="ac", bufs=2, name=f"at{b0}_{qs}")
                    for h in range(H):
                        nc.tensor.matmul(
                            at,
                            lhsT=oTn[h][:, qs * 128:qs * 128 + 128],
                            rhs=wfN[:, h * D:h * D + D],
                            start=(h == 0), stop=(h == 3),
                        )
                    rt = sb.tile([128, D], F32, tag="r1", bufs=8, name=f"r1{b0}_{qs}")
                    nc.vector.tensor_tensor(rt, at, xres_t[qs], OP.add)
                    if has_bf:
                        nc.vector.tensor_tensor(rt, rt, bfB, OP.add)
                    r1.append(rt)
                    if debug_stage == "r1":
                        ro = b0 * BS + qs * 128
                        nc.sync.dma_start(out=out_d[ro:ro + 128, :], in_=rt)
                    else:
                        hn2.append(layer_norm(rt, f"2_{b0}_{qs}",
                                              apply_on_act=True))
                return (b0, nb, r1, hn2)

            def rest_B(state):
                """h2T transposes + FFN; emitted after the NEXT chain's
                attention so the LN2 serial chain hides behind matmuls."""
                b0, nb, r1, hn2 = state
                if debug_stage == "r1":
                    return
                qw = nb * BS
                nqs = qw // 128
                # HAM keepalive: a tiny matmul gated on the FIRST LN2 apply
                # fires mid-way through the LN2 PE-idle stretch, splitting it
                # below the ~3.4us window after which the PE clock would
                # re-throttle to half rate.
                ka = ps.tile([128, 128], F32, tag="ac", bufs=2, name=f"ka{b0}")
                nc.tensor.matmul(ka, lhsT=ident, rhs=hn2[0][:, 0:128],
                                 start=True, stop=True)
                h2T = []
                for dt in range(4):
                    hps = ps.tile([128, qw], BF16, tag="ac", bufs=2,
                                  name=f"hp{b0}_{dt}")
                    for qs in range(nqs):
                        nc.tensor.transpose(
                            hps[:, qs * 128:qs * 128 + 128],
                            hn2[qs][:, dt * 128:dt * 128 + 128],
                            ident,
                        )
                    ht = sb.tile([128, qw], BF16, tag="h2T", bufs=5, name=f"h2{b0}_{dt}")
                    if dt % 2 == 0:
                        nc.vector.tensor_copy(ht, hps)
                    else:
                        nc.scalar.copy(ht, hps)
                    h2T.append(ht)

                # FFN up + gelu (retain gl tiles), then down per q-subtile
                gl = []
                for fb in range(16):
                    g = ps.tile([128, qw], F32, tag="ac", bufs=2, name=f"g{b0}_{fb}")
                    for kt in range(4):
                        nc.tensor.matmul(
                            g,
                            lhsT=wupT[:, (kt * 16 + fb) * 128:(kt * 16 + fb) * 128 + 128],
                            rhs=h2T[kt],
                            start=(kt == 0), stop=(kt == 3),
                        )
                    gt = sb.tile([128, qw], BF16, tag="gl", bufs=17,
                                 name=f"gl{b0}_{fb}")
                    nc.scalar.activation(gt, g, AF.Gelu_apprx_tanh,
                                         bias=cuB[:, fb:fb + 1])
                    gl.append(gt)
                for qs in range(nqs):
                    y = ps.tile([128, D], F32, tag="ac", bufs=2, name=f"y{b0}_{qs}")
                    for fb in range(16):
                        nc.tensor.matmul(
                            y,
                            lhsT=gl[fb][:, qs * 128:qs * 128 + 128],
                            rhs=wdnN[:, fb * D:fb * D + D],
                            start=(fb == 0), stop=(fb == 15),
                        )
                    ot = sb.tile([128, D], F32, tag="outt", bufs=3,
                                 name=f"ot{b0}_{qs}")
                    nc.vector.tensor_tensor(ot, y, r1[qs], OP.add)
                    if has_bd:
                        nc.vector.tensor_tensor(ot, ot, bdB, OP.add)
                    ro = b0 * BS + qs * 128
                    nc.sync.dma_start(out=out_d[ro:ro + 128, :], in_=ot)

            # ---------------- emission
            # Pipeline: attention(c) -> [next phase1] -> finisher(c)+rest_A(c)
            # -> [attention(c+1)] -> rest_B(c).  Each DVE/ACT serial chain
            # (softmax normalize, LN2) is emitted behind a large block of
            # independent matmuls so the in-order tensor queue never stalls.
            for _rep in range(repeat):
                kT_ring.clear(); v_ring.clear(); qT_ring.clear()
                pending = None      # (finisher, (b0, nb, oTn)) awaiting rest
                for j in range(SLAB):
                    phase1(j)
                    if pending is not None and j < SLAB - 1:
                        fin, args = pending
                        fin()
                        rest_B(rest_A(*args))
                        pending = None
                    if j >= 3 and (j % 2) == 1 and (j - 3) // 2 <= 9:
                        b0 = 2 * ((j - 3) // 2)
                        (oTn, xres_t), fin = chain_attn(b0, 2)
                        pending = (fin, (b0, 2, oTn, xres_t))
                (last_oTn, last_xres), last_fin = chain_attn(20, 1)
                fin, args = pending
                fin()
                rest_B(rest_A(*args))
                last_fin()
                rest_B(rest_A(20, 1, last_oTn, last_xres))

    nc.compile()
    return nc


# ---------------------------------------------------------------- host side
def _prep(inputs):
    import ml_dtypes
    f8 = np.float64
    BF = ml_dtypes.bfloat16
    x = np.asarray(inputs["x"], np.float32).reshape(N, D)
    gnc = np.asarray(inputs["global_norm_conditioning"], np.float32)
    mask = np.asarray(inputs["mask"])
    wq = np.asarray(inputs["wq"], np.float32)
    wk = np.asarray(inputs["wk"], np.float32)
    wv = np.asarray(inputs["wv"], np.float32)
    w_final = np.asarray(inputs["w_final"], np.float32)
    b_final = np.asarray(inputs["b_final"], np.float32)
    w_up = np.asarray(inputs["w_up"], np.float32)
    b_up = np.asarray(inputs["b_up"], np.float32)
    w_down = np.asarray(inputs["w_down"], np.float32)
    b_down = np.asarray(inputs["b_down"], np.float32)
    w_cond = np.asarray(inputs["w_cond"], np.float32)
    b_cond = np.asarray(inputs["b_cond"], np.float32)

    so = gnc.astype(f8) @ w_cond.astype(f8) + b_cond.astype(f8)
    sc = 1.0 + so[0, :D]
    off = so[0, D:]

    wq2 = wq.astype(f8) * sc[:, None] * SCALE
    cq = (off @ wq.astype(f8)) * SCALE
    wk2 = wk.astype(f8) * sc[:, None]
    ck = off @ wk.astype(f8)
    wv2 = wv.astype(f8) * sc[:, None]
    cv = off @ wv.astype(f8)
    wu2 = w_up.astype(f8) * sc[:, None]
    cu = off @ w_up.astype(f8) + b_up.astype(f8)

    def to32(a):
        return np.ascontiguousarray(a, np.float32)

    def tobf(a):
        return np.ascontiguousarray(a, np.float32).astype(BF)

    dev = {}
    dev["wqT"] = tobf(wq2.reshape(4, 128, H, HD).transpose(1, 0, 2, 3).reshape(128, -1))
    dev["wkT"] = tobf(wk2.reshape(4, 128, H, HD).transpose(1, 0, 2, 3).reshape(128, -1))
    dev["wvN"] = tobf(wv2.reshape(4, 128, D).transpose(1, 0, 2).reshape(128, -1))
    dev["wfN"] = tobf(
        w_final.astype(f8).reshape(H, HD, D).transpose(1, 0, 2).reshape(HD, -1))
    dev["wupT"] = tobf(
        wu2.reshape(4, 128, 16, 128).transpose(1, 0, 2, 3).reshape(128, -1))
    dev["wdnN"] = tobf(
        w_down.astype(f8).reshape(16, 128, D).transpose(1, 0, 2).reshape(128, -1))
    dev["cqB"] = to32(cq.reshape(H, HD).T)
    dev["ckB"] = to32(ck.reshape(H, HD).T)
    dev["cvB"] = to32(np.tile(cv[None, :], (128, 1)))
    dev["cuB"] = to32(cu.reshape(16, 128).T)
    dev["ident"] = np.eye(128, dtype=np.float32).astype(BF)
    dev["epsB"] = np.full((128, 1), EPS, np.float32)

    has_bf = bool(np.any(b_final != 0))
    has_bd = bool(np.any(b_down != 0))
    if has_bf:
        dev["bfB"] = to32(np.tile(b_final[None, :], (128, 1)))
    if has_bd:
        dev["bdB"] = to32(np.tile(b_down[None, :], (128, 1)))

    # global key validity from the diagonal mask (keys of block n)
    kv_global = np.asarray(mask[0, 0, :, 0, 0, :], bool).reshape(NP)

    # per-core x slab + key mask
    per_core = []
    xpad = np.zeros((NP, D), np.float32)
    xpad[:N] = x
    for c in range(NCORES):
        g0 = (STARTS[c] - 1) * BS
        xs = np.zeros((SNODES, D), np.float32)
        km = np.zeros(SNODES, np.float32)
        lo = max(0, -g0)
        hi = min(SNODES, NP - g0)
        xs[lo:hi] = xpad[g0 + lo:g0 + hi]
        kmv = np.zeros(SNODES, bool)
        kmv[lo:hi] = kv_global[g0 + lo:g0 + hi]
        km[:] = kmv.astype(np.float32)
        # sanity: invalid keys only at statically-fixed kt tiles
        km_t = kmv.reshape(KT, 128)
        for kt in range(KT):
            if not km_t[kt].all():
                assert kt in FIXUP_KTS, f"unexpected invalid keys at kt={kt}"
        per_core.append({
            "x_slab": xs,
            "kmask": np.ascontiguousarray(kmv.reshape(KT, 128).T.astype(np.float32)),
        })
    return dev, per_core, has_bf, has_bd


def _run(inputs, trace=False, trace_kwargs=None):
    from concourse.bass_utils import run_bass_kernel_spmd

    import os
    dbg = os.environ.get("KERNEL_DEBUG_STAGE") or None
    rep = int(os.environ.get("KERNEL_REPEAT", "1"))
    dev, per_core, has_bf, has_bd = _prep(inputs)
    key = (has_bf, has_bd, dbg, rep)
    if key not in _PROG_CACHE:
        _PROG_CACHE[key] = _build_program(has_bf, has_bd, debug_stage=dbg,
                                          repeat=rep)
    nc = _PROG_CACHE[key]

    in_maps = []
    for c in range(NCORES):
        m = dict(dev)
        m.update(per_core[c])
        in_maps.append(m)
    kw = {}
    if trace:
        kw["trace"] = True
        if trace_kwargs:
            kw.update(trace_kwargs)
    res = run_bass_kernel_spmd(nc, in_maps, list(range(NCORES)), **kw)

    out = np.zeros((NP, D), np.float32)
    for c in range(NCORES):
        nblk = NB - STARTS[c] if c == NCORES - 1 else STARTS[c + 1] - STARTS[c]
        rows = nblk * BS
        out[STARTS[c] * BS: STARTS[c] * BS + rows] = res.results[c]["out"][:rows]
    x_in = np.asarray(inputs["x"])
    return out[:N].reshape(1, N, D).astype(x_in.dtype), res


def kernel(**inputs):
    out, _ = _run(inputs)
    return out
